# revision 35
# baseline (speedup 1.0000x reference)
"""CARAFE-naive upsampling (N=4, C=256, H=W=64, k=5, g=4, s=2) on 8 TRN2
NeuronCores.

Strategy
--------
Sharding: core c <- (batch n = c//2, group-pair j = c%2). Each core owns 128
feature channels (2 of the 4 mask groups) of one batch image.

Compute: blocked im2col. The output is tiled into 4x8 source blocks; a
block's 25-tap neighborhood lives in an 8x12 source window (K=96). Per
(tile, group) ONE matmul computes every tap in a single pass:

    psum[(h',a,w,b), c] = sum_{(r,w'')} statT[(r,w''), (h',a,w,b)]
                                      * feat[(r,w''), c]

statT is the host-sheared mask tile (each column holds one output pixel's
25 taps placed at its window offsets; 96/25 = 3.8x inflation over raw
masks) and feat the host-im2col'd feature window, both in f8e3m4; output
goes back as f16. Every psum element is real output: full [128, 512]
PSUM banks drain with one contiguous 128-partition DVE/ACT copy per
(row-block, group).

Dataflow: everything is SBUF-resident. DRAM layouts are hb-major inside
each partition row so load slabs are 2-8KB contiguous per partition
(reads are DMA-engine latency-bound; fat packets amortize it). Loads
ride the sync ring alone, stores the scalar ring alone -- mixing
directions in one HWDGE ring halves its throughput. Slab sizes ramp up
(1,1,2,4,4,4 hb) so matmuls start ~1us after the first slab; stores
drain a persistent out buffer in 4,4,4,2,2-hb slices so the final store
tail is thin. Total HBM traffic 8.9 MB/core (3.15 stat + 1.57 feat +
4.19 out) against a ~260 GB/s read / ~420 GB/s write roofline.
"""

import sys

import numpy as np
from numpy.lib.stride_tricks import sliding_window_view

for _p in ("/opt/trn_rl_repo", "/opt/pypackages"):
    if _p not in sys.path:
        sys.path.append(_p)

import ml_dtypes  # noqa: E402
from contextlib import ExitStack  # noqa: E402

import concourse.bass as bass  # noqa: E402
import concourse.tile as tile  # noqa: E402
from concourse import bacc, mybir  # noqa: E402
from concourse.bass_utils import run_bass_kernel_spmd  # noqa: E402

# Problem constants (hardcoded per harness contract)
N, C, H, W = 4, 256, 64, 64
NB = 16          # row blocks (4 source rows each)
NWP = 8          # col blocks (8 source cols each)
K = 96           # contraction = 8x12 source window
KP = 128         # SBUF-side K pad to 128: turns on FWL's LDWEIGHTS/MATMUL
                 # overlap (halves the PE pair rate). Pad rows are zeroed
                 # on-device (memset seed + SBUF->SBUF broadcast DMA), so
                 # the wire still carries only 96 rows.
F8E3 = ml_dtypes.float8_e3m4

# load slab hb-ranges (ramp up, then back down so the last slab's
# residual compute tail is short). Kept to 14 descriptors: Tile rotates
# 8 HW-DMA completion semaphores, so issue k waits for descriptor k-8
# to fully retire -- late issues must pair with early (tiny) slabs.
LOAD_SLABS = [(0, 1), (1, 3), (3, 6), (6, 10), (10, 13), (13, 15),
              (15, 16)]
# store slab hb-ranges: small (2-hb, 524KB) slabs issued at production
# cadence so the write stream trickles instead of bursting -- big store
# descriptors monopolize the DMA engines and starve the load stream.
# The tail batches 4 hb: by then loads are done and one fat descriptor
# drains fastest.
STORE_SLABS = [(0, 2), (2, 4), (4, 6), (6, 8), (8, 10), (10, 12),
               (12, 16)]

_NC_CACHE = {}


def _build_bass():
    nc = bacc.Bacc()
    # hb-major-per-partition layouts: a slab covering hb range [a, b) is
    # (b-a)*2KB (stat) / (b-a)*1KB (feat) contiguous per partition row
    # stat (2048B/hb) and feat (1024B/hb) concatenated per (row, hb):
    # one descriptor per load slab, 3KB-per-hb contiguous partition runs
    blob_d = nc.declare_dram_parameter(
        "blob", [K, NB, 3 * NWP * 128], mybir.dt.float8e3, isOutput=False)
    out_d = nc.declare_dram_parameter(
        "out", [128, NB, 2, NWP, 64], mybir.dt.float16, isOutput=True)

    with tile.TileContext(nc) as tc, ExitStack() as ctx:
        sp = ctx.enter_context(tc.tile_pool(name="sp", bufs=1))
        pp = ctx.enter_context(tc.tile_pool(name="pp", bufs=8, space="PSUM"))

        # whole problem resident in SBUF: blob (stat+feat) 48KB/part,
        # out 32KB/part. Free dims pre-merged so every DMA view is <=3
        # dims (the AP balancer can't merge >3 dims across 0-strides).
        blob_sb = sp.tile([KP, NB, 3 * NWP * 128], mybir.dt.float8e3,
                          name="blob_sb", tag="bl")
        out_sb = sp.tile([128, NB, 2, NWP, 64], mybir.dt.float16,
                         name="out_sb", tag="ot")

        # K-pad rows 96..127 (zeros that enable FWL): gpsimd (boots ~6.4us
        # and is otherwise idle) zeroes them as u32-bitcast memsets (4x
        # fewer elements than fp8), fine-grained by hb range so each
        # slab's matmuls only gate on their own pad chunk. ~8.6us total,
        # always ahead of the load stream.
        for a, b in [(0, 1), (1, 4), (4, 8), (8, 12), (12, 16)]:
            nc.gpsimd.memset(
                blob_sb[K:, a:b].bitcast(mybir.dt.uint32), 0)

        # loads ride the sync HWDGE ring alone (mixed-direction rings
        # halve their throughput); the wire carries only the 96 real K
        # rows
        for a, b in LOAD_SLABS:
            nc.sync.dma_start(out=blob_sb[:K, a:b], in_=blob_d[:, a:b])

        si = 0
        for hb in range(NB):
            for g in range(2):
                ps = pp.tile([128, NWP, 64], mybir.dt.float32,
                             name=f"p{hb}_{g}", tag="ps")
                for wbp in range(NWP):
                    co = (g * NWP + wbp) * 128
                    fo = 2 * NWP * 128 + 128 * wbp + 64 * g
                    nc.tensor.matmul(
                        out=ps[:, wbp, :],
                        lhsT=blob_sb[:, hb, co: co + 128],
                        rhs=blob_sb[:, hb, fo: fo + 64],
                        start=True, stop=True,
                        skip_group_check=True,
                    )
                # drain the full bank with one contiguous 128-partition
                # copy; vector/scalar split keeps pace with the PE
                if g == 0:
                    nc.vector.tensor_copy(out=out_sb[:, hb, g], in_=ps)
                else:
                    nc.scalar.copy(out=out_sb[:, hb, g], in_=ps)
            # stores ride the scalar ring alone, issued right after the
            # slab's last copy (splitting them onto the sync ring behind
            # the loads measured slower -- the free-running overlap wins)
            if si < len(STORE_SLABS) and hb == STORE_SLABS[si][1] - 1:
                a, b = STORE_SLABS[si]
                nc.scalar.dma_start(out=out_d[:, a:b], in_=out_sb[:, a:b])
                si += 1

    nc.finalize()
    return nc


def _host_shards(features, masks):
    """Build per-core stat/feat arrays (f8e3m4)."""
    in_maps = []
    for core in range(8):
        n, j = core // 2, core % 2
        f = features[n, 128 * j: 128 * (j + 1)]        # [128, 64, 64] f32
        m = masks[n, 50 * j: 50 * j + 50].reshape(2, 25, 128, 128)

        # feature im2col: feat[(r,w''), hb, wbp, c] = Fpad[c, 4hb+r, 8wbp+w'']
        fpad = np.pad(f, ((0, 0), (2, 2), (2, 2)))
        sw = sliding_window_view(fpad, (8, 12), axis=(1, 2))[:, ::4, ::8]
        feat = np.ascontiguousarray(
            sw.transpose(3, 4, 1, 2, 0)).reshape(K, NB, NWP * 128)

        # mask shear: stat[(r,w''), hb, g, wbp, (h',a,w,b)] holds tap
        # (di=r-h', dj=w''-w) of output pixel (2(4hb+h')+a, 2(8wbp+w)+b)
        mm = m.reshape(2, 5, 5, NB, 4, 2, NWP, 8, 2)  # g,di,dj,hb,h,a,wbp,w,b
        stat = np.zeros((8, 12, NB, 2, NWP, 4, 2, 8, 2), np.float32)
        for di in range(5):
            for dj in range(5):
                for hp in range(4):
                    for w in range(8):
                        stat[hp + di, w + dj, :, :, :, hp, :, w, :] = \
                            mm[:, di, dj, :, hp, :, :, w, :].transpose(
                                1, 0, 3, 2, 4)
        stat = stat.reshape(K, NB, 2 * NWP * 128)

        blob = np.concatenate([stat, feat], axis=2)
        in_maps.append({"blob": blob.astype(F8E3)})
    return in_maps


def kernel(features, masks, _trace=False):
    features = np.asarray(features, dtype=np.float32)
    masks = np.asarray(masks, dtype=np.float32)

    in_maps = _host_shards(features, masks)

    if "nc" not in _NC_CACHE:
        _NC_CACHE["nc"] = _build_bass()
    nc = _NC_CACHE["nc"]

    res = run_bass_kernel_spmd(nc, in_maps, list(range(8)), trace=_trace)
    kernel._last_result = res

    out = np.empty((N, C, 2 * H, 2 * W), np.float32)
    for core in range(8):
        n, j = core // 2, core % 2
        od = res.results[core]["out"].astype(np.float32)
        od = od.reshape(4, 2, 8, 2, NB, 2, NWP, 64)  # h',a,w,b,hb,g,wbp,cc
        od = od.transpose(5, 7, 4, 0, 1, 6, 2, 3)    # g,cc,hb,h',a,wbp,w,b
        out[n, 128 * j: 128 * (j + 1)] = od.reshape(128, 128, 128)
    return out


# revision 38
# speedup vs baseline: 1.0546x; 1.0546x over previous
"""CARAFE-naive upsampling (N=4, C=256, H=W=64, k=5, g=4, s=2) on 8 TRN2
NeuronCores.

Strategy
--------
Sharding: core c <- (batch n = c//2, group-pair j = c%2). Each core owns 128
feature channels (2 of the 4 mask groups) of one batch image.

Compute: blocked im2col. The output is tiled into 4x8 source blocks; a
block's 25-tap neighborhood lives in an 8x12 source window (K=96). Per
(tile, group) ONE matmul computes every tap in a single pass:

    psum[(h',a,w,b), c] = sum_{(r,w'')} statT[(r,w''), (h',a,w,b)]
                                      * feat[(r,w''), c]

statT is the host-sheared mask tile (each column holds one output pixel's
25 taps placed at its window offsets; 96/25 = 3.8x inflation over raw
masks) and feat the host-im2col'd feature window, both in f8e3m4 and
concatenated into one "blob" tensor (one DMA descriptor per slab, 3KB
contiguous per partition per row-block); output goes back as f16. Every
psum element is real output: full [128, 512] PSUM banks drain with one
contiguous 128-partition DVE/ACT copy per (row-block, group).

The contraction is zero-padded to KP=128 in SBUF only: full-height
weights turn on the PE's fast-weight-load overlap, which hides
LDWEIGHTS under MATMUL and halves the PE stream from ~26us to ~12us.
The pad rows are zeroed by gpsimd u32 memsets (idle engine, boots
early), hb-range-sliced so each slab's matmuls gate only on their own
chunk; the wire still carries only the 96 real rows.

Dataflow: everything is SBUF-resident. DRAM layouts are hb-major inside
each partition row so load slabs are 3-12KB contiguous per partition.
Loads ride the sync ring alone, stores the scalar ring alone -- mixing
directions in one HWDGE ring halves its throughput, and Tile's 8-deep
HW-DMA-semaphore rotation means late issues must pair with early tiny
slabs. Stores go as 2-hb slices at production cadence (big store
descriptors monopolize the DMA engines and starve the load stream).
Total HBM traffic 8.9 MB/core (3.15 stat + 1.57 feat + 4.19 out)
against a ~260 GB/s read / ~420 GB/s 2-queue write roofline; measured
~40.8us vs ~10.5us of fixed NEFF boot/teardown overhead.
"""

import sys

import numpy as np
from numpy.lib.stride_tricks import sliding_window_view

for _p in ("/opt/trn_rl_repo", "/opt/pypackages"):
    if _p not in sys.path:
        sys.path.append(_p)

import ml_dtypes  # noqa: E402
from contextlib import ExitStack  # noqa: E402

import concourse.bass as bass  # noqa: E402
import concourse.tile as tile  # noqa: E402
from concourse import bacc, mybir  # noqa: E402
from concourse.bass_utils import run_bass_kernel_spmd  # noqa: E402

# Problem constants (hardcoded per harness contract)
N, C, H, W = 4, 256, 64, 64
NB = 16          # row blocks (4 source rows each)
NWP = 8          # col blocks (8 source cols each)
K = 96           # contraction = 8x12 source window
KP = 128         # SBUF-side K pad to 128: turns on FWL's LDWEIGHTS/MATMUL
                 # overlap (halves the PE pair rate). Pad rows are zeroed
                 # on-device (memset seed + SBUF->SBUF broadcast DMA), so
                 # the wire still carries only 96 rows.
F8E3 = ml_dtypes.float8_e3m4

# load slab hb-ranges (ramp up, then back down so the last slab's
# residual compute tail is short). Kept to 14 descriptors: Tile rotates
# 8 HW-DMA completion semaphores, so issue k waits for descriptor k-8
# to fully retire -- late issues must pair with early (tiny) slabs.
LOAD_SLABS = [(0, 1), (1, 3), (3, 6), (6, 10), (10, 13), (13, 15),
              (15, 16)]
# store slab hb-ranges: small (2-hb, 524KB) slabs issued at production
# cadence so the write stream trickles at ~230 GB/s instead of bursting
# -- big store descriptors monopolize the DMA engines and starve the
# load stream. Last pair split for a thin tail.
STORE_SLABS = [(0, 2), (2, 4), (4, 6), (6, 8), (8, 10), (10, 12),
               (12, 14), (14, 15), (15, 16)]

_NC_CACHE = {}


def _build_bass():
    nc = bacc.Bacc()
    # hb-major-per-partition layouts: a slab covering hb range [a, b) is
    # (b-a)*2KB (stat) / (b-a)*1KB (feat) contiguous per partition row
    # stat (2048B/hb) and feat (1024B/hb) concatenated per (row, hb):
    # one descriptor per load slab, 3KB-per-hb contiguous partition runs
    blob_d = nc.declare_dram_parameter(
        "blob", [K, NB, 3 * NWP * 128], mybir.dt.float8e3, isOutput=False)
    out_d = nc.declare_dram_parameter(
        "out", [128, NB, 2, NWP, 64], mybir.dt.float16, isOutput=True)

    with tile.TileContext(nc) as tc, ExitStack() as ctx:
        sp = ctx.enter_context(tc.tile_pool(name="sp", bufs=1))
        pp = ctx.enter_context(tc.tile_pool(name="pp", bufs=8, space="PSUM"))

        # whole problem resident in SBUF: blob (stat+feat) 48KB/part,
        # out 32KB/part. Free dims pre-merged so every DMA view is <=3
        # dims (the AP balancer can't merge >3 dims across 0-strides).
        blob_sb = sp.tile([KP, NB, 3 * NWP * 128], mybir.dt.float8e3,
                          name="blob_sb", tag="bl")
        out_sb = sp.tile([128, NB, 2, NWP, 64], mybir.dt.float16,
                         name="out_sb", tag="ot")

        # K-pad rows 96..127 (zeros that enable FWL): gpsimd (boots ~6.4us
        # and is otherwise idle) zeroes them as u32-bitcast memsets (4x
        # fewer elements than fp8), fine-grained by hb range so each
        # slab's matmuls only gate on their own pad chunk. ~8.6us total,
        # always ahead of the load stream.
        for a, b in [(0, 1), (1, 4), (4, 8), (8, 12), (12, 16)]:
            nc.gpsimd.memset(
                blob_sb[K:, a:b].bitcast(mybir.dt.uint32), 0)

        # loads ride the sync HWDGE ring alone (mixed-direction rings
        # halve their throughput); the wire carries only the 96 real K
        # rows
        for a, b in LOAD_SLABS:
            nc.sync.dma_start(out=blob_sb[:K, a:b], in_=blob_d[:, a:b])

        si = 0
        for hb in range(NB):
            for g in range(2):
                ps = pp.tile([128, NWP, 64], mybir.dt.float32,
                             name=f"p{hb}_{g}", tag="ps")
                for wbp in range(NWP):
                    co = (g * NWP + wbp) * 128
                    fo = 2 * NWP * 128 + 128 * wbp + 64 * g
                    nc.tensor.matmul(
                        out=ps[:, wbp, :],
                        lhsT=blob_sb[:, hb, co: co + 128],
                        rhs=blob_sb[:, hb, fo: fo + 64],
                        start=True, stop=True,
                        skip_group_check=True,
                    )
                # drain the full bank with one contiguous 128-partition
                # copy; vector/scalar split keeps pace with the PE.
                # Scalar also runs the store issues, so the last blocks'
                # copies go to the (less loaded) vector engine to keep
                # the drain off the critical tail.
                if g == 0 or hb >= 12:
                    nc.vector.tensor_copy(out=out_sb[:, hb, g], in_=ps)
                else:
                    nc.scalar.copy(out=out_sb[:, hb, g], in_=ps)
            # stores ride the scalar ring alone, issued right after the
            # slab's last copy (splitting them onto the sync ring behind
            # the loads measured slower -- the free-running overlap wins)
            if si < len(STORE_SLABS) and hb == STORE_SLABS[si][1] - 1:
                a, b = STORE_SLABS[si]
                nc.scalar.dma_start(out=out_d[:, a:b], in_=out_sb[:, a:b])
                si += 1

    nc.finalize()
    return nc


def _host_shards(features, masks):
    """Build per-core stat/feat arrays (f8e3m4)."""
    in_maps = []
    for core in range(8):
        n, j = core // 2, core % 2
        f = features[n, 128 * j: 128 * (j + 1)]        # [128, 64, 64] f32
        m = masks[n, 50 * j: 50 * j + 50].reshape(2, 25, 128, 128)

        # feature im2col: feat[(r,w''), hb, wbp, c] = Fpad[c, 4hb+r, 8wbp+w'']
        fpad = np.pad(f, ((0, 0), (2, 2), (2, 2)))
        sw = sliding_window_view(fpad, (8, 12), axis=(1, 2))[:, ::4, ::8]
        feat = np.ascontiguousarray(
            sw.transpose(3, 4, 1, 2, 0)).reshape(K, NB, NWP * 128)

        # mask shear: stat[(r,w''), hb, g, wbp, (h',a,w,b)] holds tap
        # (di=r-h', dj=w''-w) of output pixel (2(4hb+h')+a, 2(8wbp+w)+b)
        mm = m.reshape(2, 5, 5, NB, 4, 2, NWP, 8, 2)  # g,di,dj,hb,h,a,wbp,w,b
        stat = np.zeros((8, 12, NB, 2, NWP, 4, 2, 8, 2), np.float32)
        for di in range(5):
            for dj in range(5):
                for hp in range(4):
                    for w in range(8):
                        stat[hp + di, w + dj, :, :, :, hp, :, w, :] = \
                            mm[:, di, dj, :, hp, :, :, w, :].transpose(
                                1, 0, 3, 2, 4)
        stat = stat.reshape(K, NB, 2 * NWP * 128)

        blob = np.concatenate([stat, feat], axis=2)
        in_maps.append({"blob": blob.astype(F8E3)})
    return in_maps


def kernel(features, masks, _trace=False):
    features = np.asarray(features, dtype=np.float32)
    masks = np.asarray(masks, dtype=np.float32)

    in_maps = _host_shards(features, masks)

    if "nc" not in _NC_CACHE:
        _NC_CACHE["nc"] = _build_bass()
    nc = _NC_CACHE["nc"]

    res = run_bass_kernel_spmd(nc, in_maps, list(range(8)), trace=_trace)
    kernel._last_result = res

    out = np.empty((N, C, 2 * H, 2 * W), np.float32)
    for core in range(8):
        n, j = core // 2, core % 2
        od = res.results[core]["out"].astype(np.float32)
        od = od.reshape(4, 2, 8, 2, NB, 2, NWP, 64)  # h',a,w,b,hb,g,wbp,cc
        od = od.transpose(5, 7, 4, 0, 1, 6, 2, 3)    # g,cc,hb,h',a,wbp,w,b
        out[n, 128 * j: 128 * (j + 1)] = od.reshape(128, 128, 128)
    return out
